# revision 52
# baseline (speedup 1.0000x reference)
# Distributed causal self-attention for 8 Trainium2 NeuronCores.
#
# Problem: B=2, T=2048, C=768, H=12 heads, D=64. y = proj(attn(qkv(x))).
#
# Sharding: 2 (batch) x 4 (head-groups of 3 heads). Core c handles batch
# c//4 and heads (c%4)*3 .. +3. Each core computes its slice of the QKV
# projection, full attention for its 3 heads, and a partial output
# projection y_part = O_heads @ Wp_slice.T. Host sums the 4 partials per
# batch and adds b_proj.
#
# Device-side structure (single flat pipeline, PE-bound):
#   - xt is host-pre-tiled to [128, c(4) k(6) t(512)] so each 512-col
#     chunk of all six K-tiles is one contiguous-per-partition DMA; the
#     two chunks that gate the first matmul lead the two HWDGE rings.
#   - QKV emits per-head [q|k] packed M=128 outputs in [128,512] chunks;
#     q,k are then row-duplicated (rows 64:128 = rows 0:64) via SBUF->SBUF
#     DMA so attention S-matmuls (K=64) can be issued in PAIRS to PE row
#     groups (0,0) and (64,0) via tile_position -> both halves of the
#     128x128 array compute two score tiles CONCURRENTLY (~1.6x S).
#   - Each pair-half writes its own [128,1024] sp tile from a 2-deep PSUM
#     pool and gets its own exp: the next pair's S-matmuls only WAR-wait
#     on the PREVIOUS pair's first exp, which is long done -> the
#     PE-ScalarE ping-pong never serializes.  Causal masks of diagonal
#     blocks multiply ex on GPSIMD (keeps the DVE queue free of
#     exp-dependent head-of-line blocking).
#   - QKV for heads 1-2, v-projection units, norm chains and proj tiles
#     are emitted as PE filler INSIDE the attention stream (prerequisite-
#     driven: a block forces its head's qk+dup first, each O-pair its v
#     tiles), so the ScalarE-heavy attention overlaps all projections.
#     v8-15 and proj tiles are saved to pace the exp-bound p1 phase.
#   - Pass-major block order; p1 pairs are (odd,even) so the fuller tile
#     rides row group 64 and per-pass norms fire before the block ends.
#   - ~38 K=128 dummy matmuls warm the PE HAM clock gate during the
#     otherwise-dead input-DMA window (K=1 matmuls do NOT count as PE
#     activity); tail proj tiles reuse the freed sp banks as [128,1024]
#     tiles with a single alternating-engine cast.
#   PSUM: sp [128,1024]x2 (4 banks) + ot [128,1024] (2) + fill
#   [128,512]x2 (2, shared by qkv/v/proj/norm-bs units) = all 8 banks.

import numpy as np

B, T, C, H, D = 2, 2048, 768, 12, 64
HPG = 3                      # heads per group
G = 4                        # head groups
CPG = HPG * D                # 192 channels per group
KT = C // 128                # 6 contraction tiles for projections
NT = T // 128                # 16 seq tiles
PW = 1024                    # tq pass width
SCALE = float(1.0 / np.sqrt(2.0))   # 1/sqrt(B) (faithful to reference)

_CACHE = {}


def _build_module():
    import concourse.bass as bass
    import concourse.tile as tile
    import concourse.mybir as mybir
    from concourse.bacc import Bacc
    from contextlib import ExitStack

    f32 = mybir.dt.float32
    bf16 = mybir.dt.bfloat16
    AF = mybir.ActivationFunctionType

    nc = Bacc()

    xt_d = nc.dram_tensor("xt", [128, 4 * KT * (T // 4)], bf16,
                          kind="ExternalInput")
    wqkt_d = nc.dram_tensor("wqkt", [C, HPG * 128], bf16, kind="ExternalInput")
    wvt_d = nc.dram_tensor("wvt", [C, CPG], bf16, kind="ExternalInput")
    bqk_d = nc.dram_tensor("bqk", [128, HPG], f32, kind="ExternalInput")
    bv_d = nc.dram_tensor("bv", [128, CPG], f32, kind="ExternalInput")
    wpt_d = nc.dram_tensor("wpt", [CPG, C], bf16, kind="ExternalInput")
    mask_d = nc.dram_tensor("mask", [128, 128], bf16, kind="ExternalInput")
    y_d = nc.dram_tensor("y", [T, C], bf16, kind="ExternalOutput")

    with tile.TileContext(nc) as tc, ExitStack() as ctx:
        sb = ctx.enter_context(tc.tile_pool(name="sb", bufs=1))
        ps = ctx.enter_context(tc.tile_pool(name="ps", bufs=1, space="PSUM"))

        def fill_tile(name):
            return ps.tile([128, 512], f32, tag="fill", bufs=2, name=name)

        # ---- weights / constants into SBUF ----
        # wqkt + first half of xt gate the first matmul: one big DMA each
        # on the two HWDGE queues (sync, scalar).
        # xt arrives host-pre-tiled as [128, c(4) k(6) t(512)] so each
        # 512-col chunk is ONE contiguous-6KB-per-partition DMA (full
        # descriptor efficiency); chunks alternate across both HWDGE rings.
        wqkt_sb = sb.tile([128, KT * HPG * 128], bf16, tag="wqk", name="wqkt")
        nc.sync.dma_start(
            wqkt_sb[:, :].rearrange("p (k m) -> p k m", k=KT),
            wqkt_d[:, :].rearrange("(k p) m -> p k m", p=128))
        xt_sb = sb.tile([128, KT * T], bf16, tag="xt", name="xt")
        xt4 = xt_sb[:, :].rearrange("p (c k t) -> p c k t", c=4, k=KT)
        xd4 = xt_d[:, :].rearrange("p (c k t) -> p c k t", c=4, k=KT)
        CW = T // 4  # 512 cols per chunk
        # xt chunk 0 (scalar ring) and wqkt (sync ring) get the DMA fabric
        # to themselves first — they gate the first matmul.  The remaining
        # chunks go via the GPSIMD SWDGE queue, whose slower descriptor
        # generation naturally defers them.
        nc.scalar.dma_start(xt4[:, 0], xd4[:, 0])
        nc.scalar.dma_start(xt4[:, 2], xd4[:, 2])
        nc.sync.dma_start(xt4[:, 1], xd4[:, 1])
        nc.sync.dma_start(xt4[:, 3], xd4[:, 3])
        wvt_sb = sb.tile([128, KT * CPG], bf16, tag="wv", name="wvt")
        nc.sync.dma_start(
            wvt_sb[:, :].rearrange("p (k m) -> p k m", k=KT),
            wvt_d[:, :].rearrange("(k p) m -> p k m", p=128))
        bqk_sb = sb.tile([128, HPG], f32, tag="bqk", name="bqk")
        nc.scalar.dma_start(bqk_sb[:, :], bqk_d[:, :])
        bv_sb = sb.tile([128, CPG], f32, tag="bv", name="bv")
        nc.scalar.dma_start(bv_sb[:, :], bv_d[:, :])
        mask_sb = sb.tile([128, 128], bf16, tag="mask", name="mask")
        nc.gpsimd.dma_start(mask_sb[:, :], mask_d[:, :])
        wpt0_sb = sb.tile([128, C], bf16, tag="wpt0", name="wpt0")
        nc.gpsimd.dma_start(wpt0_sb[:, :], wpt_d[0:128, :])
        # K-pad second proj K-tile to 128 rows of zeros (full PE array).
        wpt1_sb = sb.tile([128, C], bf16, tag="wpt1", name="wpt1")
        nc.vector.memset(wpt1_sb[64:128, :], 0.0)
        nc.gpsimd.dma_start(wpt1_sb[0:64, :], wpt_d[128:CPG, :])
        ones_sb = sb.tile([1, 128], bf16, tag="ones", name="ones")
        nc.vector.memset(ones_sb[:, :], 1.0)
        # Warm the ScalarE exp spline table before attention needs it.
        expwarm = sb.tile([1, 128], f32, tag="expwarm", name="expwarm")
        nc.scalar.activation(expwarm[:, :], ones_sb[:, :], AF.Exp)
        # Warm the PE HAM clock gate during the otherwise-dead input-DMA
        # window: ~38 dummy K=128 matmuls keep the PE busy from ~8us until
        # the first real qk matmul (~19us), so QKV runs at 8/8 clock
        # instead of paying the cold 4/8 rate (and the MID-window idle
        # re-throttle).  Results land in one fill slot and are never read.
        dum = sb.tile([128, 512], bf16, tag="dum", name="dum")
        nc.vector.memset(dum[:, :], 1.0)

        # v storage: one big tile, [v(64) | ones(1) | zeros(63)] per
        # (token-tile, head); pads pre-set ONCE with two strided memsets
        # (on GPSIMD *after* its SWDGE queue has generated the input DMAs).
        vall = sb.tile([128, NT * HPG * 128], bf16, tag="vall", name="vall")
        v4 = vall[:, :].rearrange("p (t h u) -> p (t h) u", h=HPG, u=128)
        nc.gpsimd.memset(v4[:, :, 65:128], 0.0)
        nc.vector.memset(v4[:, :, 64:65], 1.0)

        # ---- QKV q/k: per-head packed [q(64) | k(64)] outputs ----
        qk_sb = []      # [128,T]: rows 0:64 q_h, 64:128 k_h (one eviction)
        qq_sb = []      # [128,T]: q_h duplicated to both row halves
        kk_sb = []      # [128,T]: k_h duplicated
        for h in range(HPG):
            qk_sb.append(sb.tile([128, T], bf16, tag=f"qk{h}", name=f"qk{h}"))
            qq_sb.append(sb.tile([128, T], bf16, tag=f"qq{h}", name=f"qq{h}"))
            kk_sb.append(sb.tile([128, T], bf16, tag=f"kk{h}", name=f"kk{h}"))

        def qk_unit(h, c):
            """q,k for head h, cols c:c+512 -> qq_sb[h][0:64], kk_sb[h][64:]."""
            pq = fill_tile(f"pq{h}_{c}")
            for k in range(KT):
                nc.tensor.matmul(
                    pq[:, 0:512],
                    lhsT=wqkt_sb[:, k * (HPG * 128) + h * 128:
                                 k * (HPG * 128) + (h + 1) * 128],
                    rhs=xt4[:, c // CW, k, :],
                    start=(k == 0), stop=(k == KT - 1),
                )
            # ONE packed eviction per unit: halves the fill-slot WAR
            # latency vs separate q/k evictions (both were DVE-serial)
            nc.vector.tensor_scalar_add(
                qk_sb[h][:, c:c + 512], pq[:, 0:512], bqk_sb[:, h:h + 1])

        def dup_unit(h, lo, hi):
            """Build qq/kk (duplicated row halves) from the packed qk tile,
            one column half at a time: the lo half is issued right after
            the first two evictions, so pass-0 S-matmuls (which only read
            cols 0:1024) start ~4us earlier.  qq on sync / kk on scalar —
            both rings are past their input transfers by then."""
            nc.sync.dma_start(qq_sb[h][0:64, lo:hi], qk_sb[h][0:64, lo:hi])
            nc.sync.dma_start(qq_sb[h][64:128, lo:hi], qk_sb[h][0:64, lo:hi])
            nc.scalar.dma_start(kk_sb[h][0:64, lo:hi],
                                qk_sb[h][64:128, lo:hi])
            nc.scalar.dma_start(kk_sb[h][64:128, lo:hi],
                                qk_sb[h][64:128, lo:hi])

        def v_unit(t):
            pv = fill_tile(f"pv{t}")
            c, sub = t // 4, t % 4
            for k in range(KT):
                nc.tensor.matmul(
                    pv[:, 0:CPG],
                    lhsT=xt4[:, c, k, sub * 128:(sub + 1) * 128],
                    rhs=wvt_sb[:, k * CPG:(k + 1) * CPG],
                    start=(k == 0), stop=(k == KT - 1),
                )
            nc.vector.tensor_add(
                v4[:, t * HPG:(t + 1) * HPG, 0:64],
                pv[:, 0:CPG].rearrange("p (h d) -> p h d", d=64),
                bv_sb[:, :].rearrange("p (h d) -> p h d", d=64),
            )

        # ---- attention blocks: (h, p), pass-major ----
        pt0 = sb.tile([128, T], bf16, tag="pt0", name="pt0")
        pt1 = sb.tile([128, T], bf16, tag="pt1", name="pt1")
        nc.gpsimd.memset(pt1[64:128, :], 0.0)
        p_slices = [(pt0, 0), (pt0, 64), (pt1, 0)]

        # pair schedule per pass: (A, B) tile indices; B occupies row group
        # 64 and sp cols 1024:2048. B is always the fuller tile.
        pairs_p = {
            0: [(1, 0), (3, 2), (5, 4), (7, 6)],
            1: [(1, 0), (3, 2), (5, 4), (7, 6),
                (9, 8), (11, 10), (13, 12), (15, 14)],
        }

        pending = []            # deferred small stages (run off PE path)

        def drain(n=99):
            for _ in range(min(n, len(pending))):
                pending.pop(0)()

        class Block:
            def __init__(self, h, p):
                self.h, self.p = h, p
                self.base = p * PW
                self.i_max = (self.base + PW) // 128
                self.ot = None
                self.done = set()      # tiles with O emitted
                self.started = set()   # psum banks of ot with first write
                self.normed = set()
                self.last = {0: min(self.i_max - 1, self.base // 128 + 3),
                             512: min(self.i_max - 1,
                                      (self.base + 512) // 128 + 3)}

            def get_ot(self):
                if self.ot is None:
                    self.ot = ps.tile([128, PW], f32, tag="ot", bufs=1,
                                      name=f"ot{self.h}_{self.p}")
                return self.ot

            def lo(self, i):
                return max(i * 128 - self.base, 0)

            def s_pair(self, j):
                a, b = pairs_p[self.p][j]
                ex = sb.tile([128, 2048], bf16, tag="ex", bufs=4,
                             name=f"ex{self.h}_{self.p}_{j}")
                la, lb = self.lo(a), self.lo(b)
                # Two [128,1024] sp tiles from a 2-deep pool: the next
                # pair's S-matmuls only WAR-wait on this pair's FIRST exp,
                # which completes while this pair's second half still runs.
                # B (full) half first: its long exp overlaps A's matmuls.
                for off, i, l, tp in ((1024, b, lb, 64), (0, a, la, 0)):
                    sp = ps.tile([128, 1024], f32, tag="sp", bufs=2,
                                 name=f"sp{self.h}_{self.p}_{j}_{off}")
                    kv = kk_sb[self.h][tp:tp + 64, i * 128:(i + 1) * 128]
                    qv = qq_sb[self.h]
                    for b0 in (0, 512):
                        cs, ce = max(l, b0), b0 + 512
                        if cs >= ce:
                            continue
                        nc.tensor.matmul(
                            sp[:, cs:ce],
                            lhsT=kv,
                            rhs=qv[tp:tp + 64,
                                   self.base + cs:self.base + ce],
                            start=True, stop=True,
                            tile_position=(tp, 0),
                        )
                    nc.scalar.activation(ex[:, off + l:off + 1024],
                                         sp[:, l:1024],
                                         AF.Exp, scale=SCALE)
                    r = i * 128 - self.base
                    if 0 <= r < PW:
                        nc.gpsimd.tensor_mul(ex[:, off + r:off + r + 128],
                                             ex[:, off + r:off + r + 128],
                                             mask_sb[:, :])
                return ex

            def o_pair(self, j, ex):
                a, b = pairs_p[self.p][j]
                ot = self.get_ot()
                for off, i in ((0, a), (1024, b)):
                    l = self.lo(i)
                    for b0 in (0, 512):
                        cs, ce = max(l, b0), b0 + 512
                        if cs >= ce:
                            continue
                        self.done.add((i, b0))
                        stop = all(
                            (i2, b0) in self.done
                            for i2 in range(self.last[b0] + 1))
                        nc.tensor.matmul(
                            ot[:, cs:ce],
                            lhsT=vall[:, i * (HPG * 128) + self.h * 128:
                                      i * (HPG * 128) + (self.h + 1) * 128],
                            rhs=ex[:, off + cs:off + ce],
                            start=(b0 not in self.started), stop=stop,
                        )
                        self.started.add(b0)
                        if stop and b0 not in self.normed:
                            self.normed.add(b0)
                            self.norm(b0)

            def norm(self, b0):
                # rowsum (ot row 64) -> bf16 row; deferred: ones-matmul
                # broadcast, reciprocal, multiply into pdst
                # (reciprocal_approx_* requires f32 in AND out, so the
                # recip cannot be hoisted onto the 1-row rowsum directly)
                ot = self.ot
                h, p, base = self.h, self.p, self.base
                rsb = sb.tile([1, 512], bf16, tag="rsb", bufs=4,
                              name=f"rsb{h}_{p}_{b0}")
                nc.vector.tensor_copy(rsb[:, :], ot[64:65, b0:b0 + 512])
                pdst, po = p_slices[h]

                def norm_b():
                    bs = fill_tile(f"bs{h}_{p}_{b0}")
                    nc.tensor.matmul(bs[0:64, 0:512], lhsT=ones_sb[:, 0:64],
                                     rhs=rsb[:, :], start=True, stop=True)
                    rb = sb.tile([64, 512], f32, tag="rb", bufs=2,
                                 name=f"rb{h}_{p}_{b0}")
                    nc.vector.reciprocal_approx_fast(rb[:, :], bs[0:64, 0:512])
                    nc.vector.tensor_mul(
                        pdst[po:po + 64, base + b0:base + b0 + 512],
                        ot[0:64, b0:b0 + 512], rb[:, :])
                pending.append(norm_b)

        def emit_proj_tile(t, wide=False):
            ysb = sb.tile([128, C], bf16, tag=f"ysb{t % 2}", bufs=2,
                          name=f"ysb{t}")
            if wide:
                # attention is drained: the sp banks are free, so use a
                # [128,1024] tile for BOTH chunks and a single cast that
                # alternates engines (ScalarE is idle after the last exp)
                yp = ps.tile([128, 1024], f32, tag="sp" if t % 2 else "ot",
                             bufs=2 if t % 2 else 1, name=f"ypw{t}")
                for n0, nn in ((0, 512), (512, 256)):
                    nc.tensor.matmul(yp[:, n0:n0 + nn],
                                     lhsT=pt0[:, t * 128:(t + 1) * 128],
                                     rhs=wpt0_sb[:, n0:n0 + nn],
                                     start=True, stop=False)
                    nc.tensor.matmul(yp[:, n0:n0 + nn],
                                     lhsT=pt1[:, t * 128:(t + 1) * 128],
                                     rhs=wpt1_sb[:, n0:n0 + nn],
                                     start=False, stop=True)
                if t % 2:
                    nc.scalar.copy(ysb[:, 0:C], yp[:, 0:C])
                else:
                    nc.vector.tensor_copy(ysb[:, 0:C], yp[:, 0:C])
            else:
                for n0, nn in ((0, 512), (512, 256)):
                    yp = fill_tile(f"yp{t}_{n0}")
                    nc.tensor.matmul(yp[:, 0:nn],
                                     lhsT=pt0[:, t * 128:(t + 1) * 128],
                                     rhs=wpt0_sb[:, n0:n0 + nn],
                                     start=True, stop=False)
                    nc.tensor.matmul(yp[:, 0:nn],
                                     lhsT=pt1[:, t * 128:(t + 1) * 128],
                                     rhs=wpt1_sb[:, n0:n0 + nn],
                                     start=False, stop=True)
                    nc.vector.tensor_copy(ysb[:, n0:n0 + nn], yp[:, 0:nn])
            nc.sync.dma_start(y_d[t * 128:(t + 1) * 128, :], ysb[:, 0:C])

        # ---- flat pipeline ----
        # Prerequisite-driven filler emission: before a block's first S-pair
        # its head's qk+dup must be emitted; before each O-pair, the v-units
        # it consumes. Leftover fillers dribble 1-2 per pair.
        qk_lo = [False] * HPG
        qk_hi = [False] * HPG
        v_done = [False] * NT

        # pass-0 blocks only read q/k cols 0:1024, so the hi-half qkv
        # units + dup are deferred to fillers -> first exp ~2.5us earlier
        def ensure_qk_lo(h):
            if not qk_lo[h]:
                qk_lo[h] = True
                qk_unit(h, 0)
                qk_unit(h, 512)
                dup_unit(h, 0, 1024)

        def ensure_qk_hi(h):
            ensure_qk_lo(h)
            if not qk_hi[h]:
                qk_hi[h] = True
                qk_unit(h, 1024)
                qk_unit(h, 1536)
                dup_unit(h, 1024, 2048)

        def ensure_v(tiles):
            for t in tiles:
                if not v_done[t]:
                    v_done[t] = True
                    v_unit(t)

        # p0-phase fillers: only what p0 needs (heads 1-2 qkv, v4-7).
        # v8-15 and proj tiles are saved to pace the ScalarE-bound p1 phase.
        filler_q = [lambda: ensure_qk_lo(1), lambda: ensure_qk_hi(0),
                    lambda: ensure_qk_lo(2), lambda: ensure_qk_hi(1),
                    lambda: ensure_qk_hi(2)] + \
                   [lambda t=t: ensure_v([t]) for t in range(4, NT)]

        # startup: HAM warm-up, then head-0 q,k lo-half + first v tiles
        warmp = fill_tile("warmp")
        for _ in range(38):
            nc.tensor.matmul(warmp[:, 0:512], lhsT=dum[:, 0:128],
                             rhs=dum[:, 0:512], start=True, stop=True)
        ensure_qk_lo(0)
        ensure_v(range(0, 2))   # first O-pair needs only v0,v1;
        # later v tiles are forced just-in-time by do_o's ensure_v

        blocks = [Block(h, p) for p in range(2) for h in range(HPG)]
        proj_emitted = 0

        # stream: S(j) ... [O(j-1), fillers] ... S(j+1); across blocks the
        # S stream runs `depth` pair-units ahead of the O stream.
        border = [0, 1, 2, 3, 4, 5]
        stream = [(bi, j) for bi in border
                  for j in range(len(pairs_p[blocks[bi].p]))]
        depth = 2
        ex_store = {}
        proj_ready = 0

        def emit_filler_batch(n):
            nonlocal proj_ready, proj_emitted
            for _ in range(n):
                if filler_q:
                    filler_q.pop(0)()
                elif proj_emitted < proj_ready:
                    drain()   # pt0/pt1 writers must be emitted first
                    emit_proj_tile(proj_emitted)
                    proj_emitted += 1
                else:
                    break

        def do_o(obi, oj):
            nonlocal proj_ready
            blk = blocks[obi]
            a, b = pairs_p[blk.p][oj]
            ensure_v([a, b])
            blk.o_pair(oj, ex_store.pop((obi, oj)))
            if obi == 2 and oj == len(pairs_p[0]) - 1:
                proj_ready = 8           # all heads p0 normed (after drain)
            if obi == 5 and oj == 5:
                proj_ready = 12          # h2 p1 b0=0 norm fired

        for idx, (bi, j) in enumerate(stream):
            drain()
            if blocks[bi].p == 0:
                ensure_qk_lo(blocks[bi].h)
            else:
                ensure_qk_hi(blocks[bi].h)
            ex_store[(bi, j)] = blocks[bi].s_pair(j)
            if idx >= depth:
                do_o(*stream[idx - depth])
                emit_filler_batch(2)
            else:
                emit_filler_batch(1)
        for k in range(depth):
            do_o(*stream[len(stream) - depth + k])
            drain()
            emit_filler_batch(2)
        proj_ready = 16
        drain()
        while proj_emitted < 16:
            emit_proj_tile(proj_emitted, wide=True)
            proj_emitted += 1
            drain()

    nc.finalize()
    return nc


def _get_module():
    if "nc" not in _CACHE:
        _CACHE["nc"] = _build_module()
    return _CACHE["nc"]


def make_in_maps(x, w_attn, b_attn, w_proj):
    """Host-side sharding: per-core input dicts (8 cores)."""
    import ml_dtypes
    bf16 = ml_dtypes.bfloat16
    x = np.asarray(x, dtype=np.float32)
    w_attn = np.asarray(w_attn, dtype=np.float32)
    b_attn = np.asarray(b_attn, dtype=np.float32)
    w_proj = np.asarray(w_proj, dtype=np.float32)

    # xt pre-tiled to [p, c(4) k(6) t(512)]: each 512-col chunk of all six
    # 128-row k-tiles is contiguous per partition (one efficient DMA each)
    xts = []
    for b in range(B):
        xt = np.ascontiguousarray(x[b].T)            # [768, 2048]
        xt = xt.reshape(KT, 128, 4, T // 4)          # k p c t
        xt = xt.transpose(1, 2, 0, 3).reshape(128, KT * T)
        xts.append(np.ascontiguousarray(xt).astype(bf16))
    mask = np.triu(np.ones((128, 128), np.float32)).astype(bf16)

    in_maps = []
    for c in range(8):
        b = c // G
        hg = c % G
        sl = slice(CPG * hg, CPG * (hg + 1))
        wq = w_attn[0:C][sl]          # [192, 768]
        wk = w_attn[C:2 * C][sl]
        wv = w_attn[2 * C:3 * C][sl]
        bq = b_attn[0:C][sl]
        bk = b_attn[C:2 * C][sl]
        bv = b_attn[2 * C:3 * C][sl]
        # per head h: wqkt cols h*128..+128 = [wq_h (64) | wk_h (64)]
        wqkt = np.zeros((C, HPG * 128), np.float32)
        bqk = np.zeros((128, HPG), np.float32)
        for h in range(HPG):
            wqkt[:, h * 128:h * 128 + 64] = wq[h * 64:(h + 1) * 64].T
            wqkt[:, h * 128 + 64:(h + 1) * 128] = wk[h * 64:(h + 1) * 64].T
            bqk[0:64, h] = bq[h * 64:(h + 1) * 64]
            bqk[64:128, h] = bk[h * 64:(h + 1) * 64]
        wvt = np.ascontiguousarray(wv.T).astype(bf16)             # [768, 192]
        bvb = np.ascontiguousarray(
            np.broadcast_to(bv, (128, CPG))).astype(np.float32)   # [128, 192]
        wpt = np.ascontiguousarray(w_proj[:, sl].T).astype(bf16)  # [192, 768]
        in_maps.append({
            "xt": xts[b],
            "wqkt": wqkt.astype(bf16),
            "wvt": wvt,
            "bqk": bqk,
            "bv": bvb,
            "wpt": wpt,
            "mask": mask,
        })
    return in_maps


def gather(results, b_proj):
    """Sum the 4 head-group partials per batch, add bias."""
    b_proj = np.asarray(b_proj, dtype=np.float32)
    y = np.zeros((B, T, C), np.float32)
    for c in range(8):
        y[c // G] += np.asarray(results[c]["y"], dtype=np.float32)
    y += b_proj
    return y


def run(x, w_attn, b_attn, w_proj, b_proj, trace=False, **kw):
    from concourse.bass_utils import run_bass_kernel_spmd
    nc = _get_module()
    in_maps = make_in_maps(x, w_attn, b_attn, w_proj)
    res = run_bass_kernel_spmd(nc, in_maps, list(range(8)), trace=trace, **kw)
    return gather(res.results, b_proj), res


def kernel(x, w_attn, b_attn, w_proj, b_proj):
    y, _ = run(x, w_attn, b_attn, w_proj, b_proj)
    return y


# revision 53
# speedup vs baseline: 1.1226x; 1.1226x over previous
# Distributed causal self-attention for 8 Trainium2 NeuronCores.
#
# Problem: B=2, T=2048, C=768, H=12 heads, D=64. y = proj(attn(qkv(x))).
#
# Sharding: 2 (batch) x 4 (head-groups of 3 heads). Core c handles batch
# c//4 and heads (c%4)*3 .. +3. Each core computes its slice of the QKV
# projection, full attention for its 3 heads, and a partial output
# projection y_part = O_heads @ Wp_slice.T. Host sums the 4 partials per
# batch and adds b_proj.
#
# Device-side structure (single flat pipeline, PE-bound):
#   - xt is host-pre-tiled to [128, c(4) k(6) t(512)] so each 512-col
#     chunk of all six K-tiles is one contiguous-per-partition DMA; the
#     two chunks that gate the first matmul lead the two HWDGE rings.
#   - QKV emits per-head [q|k] packed M=128 outputs in [128,512] chunks;
#     q,k are then row-duplicated (rows 64:128 = rows 0:64) via SBUF->SBUF
#     DMA so attention S-matmuls (K=64) can be issued in PAIRS to PE row
#     groups (0,0) and (64,0) via tile_position -> both halves of the
#     128x128 array compute two score tiles CONCURRENTLY (~1.6x S).
#   - Each pair-half writes its own [128,1024] sp tile from a 2-deep PSUM
#     pool and gets its own exp: the next pair's S-matmuls only WAR-wait
#     on the PREVIOUS pair's first exp, which is long done -> the
#     PE-ScalarE ping-pong never serializes.  Causal masks of diagonal
#     blocks multiply ex on GPSIMD (keeps the DVE queue free of
#     exp-dependent head-of-line blocking).
#   - QKV for heads 1-2, v-projection units, norm chains and proj tiles
#     are emitted as PE filler INSIDE the attention stream (prerequisite-
#     driven: a block forces its head's qk+dup first, each O-pair its v
#     tiles), so the ScalarE-heavy attention overlaps all projections.
#     v8-15 and proj tiles are saved to pace the exp-bound p1 phase.
#   - Pass-major block order; p1 pairs are (odd,even) so the fuller tile
#     rides row group 64 and per-pass norms fire before the block ends.
#   - ~38 K=128 dummy matmuls warm the PE HAM clock gate during the
#     otherwise-dead input-DMA window (K=1 matmuls do NOT count as PE
#     activity); tail proj tiles reuse the freed sp banks as [128,1024]
#     tiles with a single alternating-engine cast.
#   PSUM: sp [128,1024]x2 (4 banks) + ot [128,1024] (2) + fill
#   [128,512]x2 (2, shared by qkv/v/proj/norm-bs units) = all 8 banks.

import numpy as np

B, T, C, H, D = 2, 2048, 768, 12, 64
HPG = 3                      # heads per group
G = 4                        # head groups
CPG = HPG * D                # 192 channels per group
KT = C // 128                # 6 contraction tiles for projections
NT = T // 128                # 16 seq tiles
PW = 1024                    # tq pass width
SCALE = float(1.0 / np.sqrt(2.0))   # 1/sqrt(B) (faithful to reference)

_CACHE = {}


def _build_module():
    import concourse.bass as bass
    import concourse.tile as tile
    import concourse.mybir as mybir
    from concourse.bacc import Bacc
    from contextlib import ExitStack

    f32 = mybir.dt.float32
    bf16 = mybir.dt.bfloat16
    AF = mybir.ActivationFunctionType

    nc = Bacc()

    xt_d = nc.dram_tensor("xt", [128, 4 * KT * (T // 4)], bf16,
                          kind="ExternalInput")
    wqkt_d = nc.dram_tensor("wqkt", [C, HPG * 128], bf16, kind="ExternalInput")
    wvt_d = nc.dram_tensor("wvt", [C, CPG], bf16, kind="ExternalInput")
    bqk_d = nc.dram_tensor("bqk", [128, HPG], f32, kind="ExternalInput")
    bv_d = nc.dram_tensor("bv", [128, CPG], f32, kind="ExternalInput")
    wpt_d = nc.dram_tensor("wpt", [CPG, C], bf16, kind="ExternalInput")
    mask_d = nc.dram_tensor("mask", [128, 128], bf16, kind="ExternalInput")
    y_d = nc.dram_tensor("y", [T, C], bf16, kind="ExternalOutput")

    with tile.TileContext(nc) as tc, ExitStack() as ctx:
        sb = ctx.enter_context(tc.tile_pool(name="sb", bufs=1))
        ps = ctx.enter_context(tc.tile_pool(name="ps", bufs=1, space="PSUM"))

        def fill_tile(name):
            return ps.tile([128, 512], f32, tag="fill", bufs=2, name=name)

        # ---- weights / constants into SBUF ----
        # wqkt + first half of xt gate the first matmul: one big DMA each
        # on the two HWDGE queues (sync, scalar).
        # xt arrives host-pre-tiled as [128, c(4) k(6) t(512)] so each
        # 512-col chunk is ONE contiguous-6KB-per-partition DMA (full
        # descriptor efficiency); chunks alternate across both HWDGE rings.
        wqkt_sb = sb.tile([128, KT * HPG * 128], bf16, tag="wqk", name="wqkt")
        nc.sync.dma_start(
            wqkt_sb[:, :].rearrange("p (k m) -> p k m", k=KT),
            wqkt_d[:, :].rearrange("(k p) m -> p k m", p=128))
        xt_sb = sb.tile([128, KT * T], bf16, tag="xt", name="xt")
        xt4 = xt_sb[:, :].rearrange("p (c k t) -> p c k t", c=4, k=KT)
        xd4 = xt_d[:, :].rearrange("p (c k t) -> p c k t", c=4, k=KT)
        CW = T // 4  # 512 cols per chunk
        # xt chunk 0 (scalar ring) and wqkt (sync ring) get the DMA fabric
        # to themselves first — they gate the first matmul.  The remaining
        # chunks go via the GPSIMD SWDGE queue, whose slower descriptor
        # generation naturally defers them.
        nc.scalar.dma_start(xt4[:, 0], xd4[:, 0])
        nc.scalar.dma_start(xt4[:, 2], xd4[:, 2])
        nc.sync.dma_start(xt4[:, 1], xd4[:, 1])
        nc.sync.dma_start(xt4[:, 3], xd4[:, 3])
        wvt_sb = sb.tile([128, KT * CPG], bf16, tag="wv", name="wvt")
        nc.sync.dma_start(
            wvt_sb[:, :].rearrange("p (k m) -> p k m", k=KT),
            wvt_d[:, :].rearrange("(k p) m -> p k m", p=128))
        bqk_sb = sb.tile([128, HPG], f32, tag="bqk", name="bqk")
        nc.scalar.dma_start(bqk_sb[:, :], bqk_d[:, :])
        bv_sb = sb.tile([128, CPG], f32, tag="bv", name="bv")
        nc.scalar.dma_start(bv_sb[:, :], bv_d[:, :])
        mask_sb = sb.tile([128, 128], bf16, tag="mask", name="mask")
        nc.gpsimd.dma_start(mask_sb[:, :], mask_d[:, :])
        wpt0_sb = sb.tile([128, C], bf16, tag="wpt0", name="wpt0")
        nc.gpsimd.dma_start(wpt0_sb[:, :], wpt_d[0:128, :])
        # K-pad second proj K-tile to 128 rows of zeros (full PE array).
        wpt1_sb = sb.tile([128, C], bf16, tag="wpt1", name="wpt1")
        nc.vector.memset(wpt1_sb[64:128, :], 0.0)
        nc.gpsimd.dma_start(wpt1_sb[0:64, :], wpt_d[128:CPG, :])
        ones_sb = sb.tile([1, 128], bf16, tag="ones", name="ones")
        nc.vector.memset(ones_sb[:, :], 1.0)
        # Warm the ScalarE exp spline table before attention needs it.
        expwarm = sb.tile([1, 128], f32, tag="expwarm", name="expwarm")
        nc.scalar.activation(expwarm[:, :], ones_sb[:, :], AF.Exp)
        # Warm the PE HAM clock gate during the otherwise-dead input-DMA
        # window: ~38 dummy K=128 matmuls keep the PE busy from ~8us until
        # the first real qk matmul (~19us), so QKV runs at 8/8 clock
        # instead of paying the cold 4/8 rate (and the MID-window idle
        # re-throttle).  Results land in one fill slot and are never read.
        dum = sb.tile([128, 512], bf16, tag="dum", name="dum")
        nc.vector.memset(dum[:, :], 1.0)

        # v storage: one big tile, [v(64) | ones(1) | zeros(63)] per
        # (token-tile, head); pads pre-set ONCE with two strided memsets
        # (on GPSIMD *after* its SWDGE queue has generated the input DMAs).
        vall = sb.tile([128, NT * HPG * 128], bf16, tag="vall", name="vall")
        v4 = vall[:, :].rearrange("p (t h u) -> p (t h) u", h=HPG, u=128)
        nc.gpsimd.memset(v4[:, :, 65:128], 0.0)
        nc.vector.memset(v4[:, :, 64:65], 1.0)

        # ---- QKV q/k: per-head packed [q(64) | k(64)] outputs ----
        qk_sb = []      # [128,T]: rows 0:64 q_h, 64:128 k_h (one eviction)
        qq_sb = []      # [128,T]: q_h duplicated to both row halves
        kk_sb = []      # [128,T]: k_h duplicated
        for h in range(HPG):
            qk_sb.append(sb.tile([128, T], bf16, tag=f"qk{h}", name=f"qk{h}"))
            qq_sb.append(sb.tile([128, T], bf16, tag=f"qq{h}", name=f"qq{h}"))
            kk_sb.append(sb.tile([128, T], bf16, tag=f"kk{h}", name=f"kk{h}"))

        def qk_unit(h, c):
            """q,k for head h, cols c:c+512 -> qq_sb[h][0:64], kk_sb[h][64:]."""
            pq = fill_tile(f"pq{h}_{c}")
            for k in range(KT):
                nc.tensor.matmul(
                    pq[:, 0:512],
                    lhsT=wqkt_sb[:, k * (HPG * 128) + h * 128:
                                 k * (HPG * 128) + (h + 1) * 128],
                    rhs=xt4[:, c // CW, k, :],
                    start=(k == 0), stop=(k == KT - 1),
                )
            # ONE packed eviction per unit: halves the fill-slot WAR
            # latency vs separate q/k evictions (both were DVE-serial)
            nc.vector.tensor_scalar_add(
                qk_sb[h][:, c:c + 512], pq[:, 0:512], bqk_sb[:, h:h + 1])

        def dup_unit(h, lo, hi):
            """Build qq/kk (duplicated row halves) from the packed qk tile,
            one column half at a time: the lo half is issued right after
            the first two evictions, so pass-0 S-matmuls (which only read
            cols 0:1024) start ~4us earlier.  qq on sync / kk on scalar —
            both rings are past their input transfers by then."""
            nc.sync.dma_start(qq_sb[h][0:64, lo:hi], qk_sb[h][0:64, lo:hi])
            nc.sync.dma_start(qq_sb[h][64:128, lo:hi], qk_sb[h][0:64, lo:hi])
            nc.scalar.dma_start(kk_sb[h][0:64, lo:hi],
                                qk_sb[h][64:128, lo:hi])
            nc.scalar.dma_start(kk_sb[h][64:128, lo:hi],
                                qk_sb[h][64:128, lo:hi])

        def v_unit(t):
            pv = fill_tile(f"pv{t}")
            c, sub = t // 4, t % 4
            for k in range(KT):
                nc.tensor.matmul(
                    pv[:, 0:CPG],
                    lhsT=xt4[:, c, k, sub * 128:(sub + 1) * 128],
                    rhs=wvt_sb[:, k * CPG:(k + 1) * CPG],
                    start=(k == 0), stop=(k == KT - 1),
                )
            nc.vector.tensor_add(
                v4[:, t * HPG:(t + 1) * HPG, 0:64],
                pv[:, 0:CPG].rearrange("p (h d) -> p h d", d=64),
                bv_sb[:, :].rearrange("p (h d) -> p h d", d=64),
            )

        # ---- attention blocks: (h, p), pass-major ----
        pt0 = sb.tile([128, T], bf16, tag="pt0", name="pt0")
        pt1 = sb.tile([128, T], bf16, tag="pt1", name="pt1")
        nc.gpsimd.memset(pt1[64:128, :], 0.0)
        p_slices = [(pt0, 0), (pt0, 64), (pt1, 0)]

        # pair schedule per pass: (A, B) tile indices; B occupies row group
        # 64 and sp cols 1024:2048. B is always the fuller tile.
        pairs_p = {
            0: [(1, 0), (3, 2), (5, 4), (7, 6)],
            1: [(1, 0), (3, 2), (5, 4), (7, 6),
                (9, 8), (11, 10), (13, 12), (15, 14)],
        }

        pending = []            # deferred small stages (run off PE path)

        def drain(n=99):
            for _ in range(min(n, len(pending))):
                pending.pop(0)()

        class Block:
            def __init__(self, h, p):
                self.h, self.p = h, p
                self.base = p * PW
                self.i_max = (self.base + PW) // 128
                self.ot = None
                self.done = set()      # tiles with O emitted
                self.started = set()   # psum banks of ot with first write
                self.normed = set()
                self.last = {0: min(self.i_max - 1, self.base // 128 + 3),
                             512: min(self.i_max - 1,
                                      (self.base + 512) // 128 + 3)}

            def get_ot(self):
                if self.ot is None:
                    self.ot = ps.tile([128, PW], f32, tag="ot", bufs=1,
                                      name=f"ot{self.h}_{self.p}")
                return self.ot

            def lo(self, i):
                return max(i * 128 - self.base, 0)

            def s_pair(self, j):
                a, b = pairs_p[self.p][j]
                ex = sb.tile([128, 2048], bf16, tag="ex", bufs=4,
                             name=f"ex{self.h}_{self.p}_{j}")
                la, lb = self.lo(a), self.lo(b)
                # Two [128,1024] sp tiles from a 2-deep pool: the next
                # pair's S-matmuls only WAR-wait on this pair's FIRST exp,
                # which completes while this pair's second half still runs.
                # B (full) half first: its long exp overlaps A's matmuls.
                for off, i, l, tp in ((1024, b, lb, 64), (0, a, la, 0)):
                    sp = ps.tile([128, 1024], f32, tag="sp", bufs=2,
                                 name=f"sp{self.h}_{self.p}_{j}_{off}")
                    kv = kk_sb[self.h][tp:tp + 64, i * 128:(i + 1) * 128]
                    qv = qq_sb[self.h]
                    for b0 in (0, 512):
                        cs, ce = max(l, b0), b0 + 512
                        if cs >= ce:
                            continue
                        nc.tensor.matmul(
                            sp[:, cs:ce],
                            lhsT=kv,
                            rhs=qv[tp:tp + 64,
                                   self.base + cs:self.base + ce],
                            start=True, stop=True,
                            tile_position=(tp, 0),
                        )
                    nc.scalar.activation(ex[:, off + l:off + 1024],
                                         sp[:, l:1024],
                                         AF.Exp, scale=SCALE)
                    r = i * 128 - self.base
                    if 0 <= r < PW:
                        nc.gpsimd.tensor_mul(ex[:, off + r:off + r + 128],
                                             ex[:, off + r:off + r + 128],
                                             mask_sb[:, :])
                return ex

            def o_pair(self, j, ex):
                a, b = pairs_p[self.p][j]
                ot = self.get_ot()
                for off, i in ((0, a), (1024, b)):
                    l = self.lo(i)
                    for b0 in (0, 512):
                        cs, ce = max(l, b0), b0 + 512
                        if cs >= ce:
                            continue
                        self.done.add((i, b0))
                        stop = all(
                            (i2, b0) in self.done
                            for i2 in range(self.last[b0] + 1))
                        nc.tensor.matmul(
                            ot[:, cs:ce],
                            lhsT=vall[:, i * (HPG * 128) + self.h * 128:
                                      i * (HPG * 128) + (self.h + 1) * 128],
                            rhs=ex[:, off + cs:off + ce],
                            start=(b0 not in self.started), stop=stop,
                        )
                        self.started.add(b0)
                        if stop and b0 not in self.normed:
                            self.normed.add(b0)
                            self.norm(b0)

            def norm(self, b0):
                # rowsum (ot row 64) -> bf16 row; deferred: ones-matmul
                # broadcast, reciprocal, multiply into pdst
                # (reciprocal_approx_* requires f32 in AND out, so the
                # recip cannot be hoisted onto the 1-row rowsum directly)
                ot = self.ot
                h, p, base = self.h, self.p, self.base
                rsb = sb.tile([1, 512], bf16, tag="rsb", bufs=4,
                              name=f"rsb{h}_{p}_{b0}")
                nc.vector.tensor_copy(rsb[:, :], ot[64:65, b0:b0 + 512])
                pdst, po = p_slices[h]

                def norm_b():
                    bs = fill_tile(f"bs{h}_{p}_{b0}")
                    nc.tensor.matmul(bs[0:64, 0:512], lhsT=ones_sb[:, 0:64],
                                     rhs=rsb[:, :], start=True, stop=True)
                    rb = sb.tile([64, 512], f32, tag="rb", bufs=2,
                                 name=f"rb{h}_{p}_{b0}")
                    nc.vector.reciprocal_approx_fast(rb[:, :], bs[0:64, 0:512])
                    nc.vector.tensor_mul(
                        pdst[po:po + 64, base + b0:base + b0 + 512],
                        ot[0:64, b0:b0 + 512], rb[:, :])
                pending.append(norm_b)

        def emit_proj_tile(t, wide=False):
            ysb = sb.tile([128, C], bf16, tag=f"ysb{t % 2}", bufs=2,
                          name=f"ysb{t}")
            if wide:
                # attention is drained: the sp banks are free, so use a
                # [128,1024] tile for BOTH chunks and a single cast that
                # alternates engines (ScalarE is idle after the last exp)
                yp = ps.tile([128, 1024], f32, tag="sp" if t % 2 else "ot",
                             bufs=2 if t % 2 else 1, name=f"ypw{t}")
                for n0, nn in ((0, 512), (512, 256)):
                    nc.tensor.matmul(yp[:, n0:n0 + nn],
                                     lhsT=pt0[:, t * 128:(t + 1) * 128],
                                     rhs=wpt0_sb[:, n0:n0 + nn],
                                     start=True, stop=False)
                    nc.tensor.matmul(yp[:, n0:n0 + nn],
                                     lhsT=pt1[:, t * 128:(t + 1) * 128],
                                     rhs=wpt1_sb[:, n0:n0 + nn],
                                     start=False, stop=True)
                if t % 2:
                    nc.scalar.copy(ysb[:, 0:C], yp[:, 0:C])
                else:
                    nc.vector.tensor_copy(ysb[:, 0:C], yp[:, 0:C])
            else:
                for n0, nn in ((0, 512), (512, 256)):
                    yp = fill_tile(f"yp{t}_{n0}")
                    nc.tensor.matmul(yp[:, 0:nn],
                                     lhsT=pt0[:, t * 128:(t + 1) * 128],
                                     rhs=wpt0_sb[:, n0:n0 + nn],
                                     start=True, stop=False)
                    nc.tensor.matmul(yp[:, 0:nn],
                                     lhsT=pt1[:, t * 128:(t + 1) * 128],
                                     rhs=wpt1_sb[:, n0:n0 + nn],
                                     start=False, stop=True)
                    nc.vector.tensor_copy(ysb[:, n0:n0 + nn], yp[:, 0:nn])
            # tail tiles alternate output rings so the final transfers
            # (which gate the epilogue barrier) overlap on both HWDGE rings
            yeng = nc.scalar if (wide and t % 2) else nc.sync
            yeng.dma_start(y_d[t * 128:(t + 1) * 128, :], ysb[:, 0:C])

        # ---- flat pipeline ----
        # Prerequisite-driven filler emission: before a block's first S-pair
        # its head's qk+dup must be emitted; before each O-pair, the v-units
        # it consumes. Leftover fillers dribble 1-2 per pair.
        qk_lo = [False] * HPG
        qk_hi = [False] * HPG
        v_done = [False] * NT

        # pass-0 blocks only read q/k cols 0:1024, so the hi-half qkv
        # units + dup are deferred to fillers -> first exp ~2.5us earlier
        def ensure_qk_lo(h):
            if not qk_lo[h]:
                qk_lo[h] = True
                qk_unit(h, 0)
                qk_unit(h, 512)
                dup_unit(h, 0, 1024)

        def ensure_qk_hi(h):
            ensure_qk_lo(h)
            if not qk_hi[h]:
                qk_hi[h] = True
                qk_unit(h, 1024)
                qk_unit(h, 1536)
                dup_unit(h, 1024, 2048)

        def ensure_v(tiles):
            for t in tiles:
                if not v_done[t]:
                    v_done[t] = True
                    v_unit(t)

        # p0-phase fillers: only what p0 needs (heads 1-2 qkv, v4-7).
        # v8-15 and proj tiles are saved to pace the ScalarE-bound p1 phase.
        filler_q = [lambda: ensure_qk_lo(1), lambda: ensure_qk_hi(0),
                    lambda: ensure_qk_lo(2), lambda: ensure_qk_hi(1),
                    lambda: ensure_qk_hi(2)] + \
                   [lambda t=t: ensure_v([t]) for t in range(4, NT)]

        # startup: HAM warm-up, then head-0 q,k lo-half + first v tiles
        warmp = fill_tile("warmp")
        for _ in range(38):
            nc.tensor.matmul(warmp[:, 0:512], lhsT=dum[:, 0:128],
                             rhs=dum[:, 0:512], start=True, stop=True)
        ensure_qk_lo(0)
        ensure_v(range(0, 2))   # first O-pair needs only v0,v1;
        # later v tiles are forced just-in-time by do_o's ensure_v

        blocks = [Block(h, p) for p in range(2) for h in range(HPG)]
        proj_emitted = 0

        # stream: S(j) ... [O(j-1), fillers] ... S(j+1); across blocks the
        # S stream runs `depth` pair-units ahead of the O stream.
        border = [0, 1, 2, 3, 4, 5]
        stream = [(bi, j) for bi in border
                  for j in range(len(pairs_p[blocks[bi].p]))]
        depth = 2
        ex_store = {}
        proj_ready = 0

        def emit_filler_batch(n):
            nonlocal proj_ready, proj_emitted
            for _ in range(n):
                if filler_q:
                    filler_q.pop(0)()
                elif proj_emitted < proj_ready:
                    drain()   # pt0/pt1 writers must be emitted first
                    emit_proj_tile(proj_emitted)
                    proj_emitted += 1
                else:
                    break

        def do_o(obi, oj):
            nonlocal proj_ready
            blk = blocks[obi]
            a, b = pairs_p[blk.p][oj]
            ensure_v([a, b])
            blk.o_pair(oj, ex_store.pop((obi, oj)))
            if obi == 2 and oj == len(pairs_p[0]) - 1:
                proj_ready = 8           # all heads p0 normed (after drain)
            if obi == 5 and oj == 5:
                proj_ready = 12          # h2 p1 b0=0 norm fired

        for idx, (bi, j) in enumerate(stream):
            drain()
            if blocks[bi].p == 0:
                ensure_qk_lo(blocks[bi].h)
            else:
                ensure_qk_hi(blocks[bi].h)
            ex_store[(bi, j)] = blocks[bi].s_pair(j)
            if idx >= depth:
                do_o(*stream[idx - depth])
                emit_filler_batch(2)
            else:
                emit_filler_batch(1)
        for k in range(depth):
            do_o(*stream[len(stream) - depth + k])
            drain()
            emit_filler_batch(2)
        proj_ready = 16
        drain()
        while proj_emitted < 16:
            emit_proj_tile(proj_emitted, wide=True)
            proj_emitted += 1
            drain()

    nc.finalize()
    return nc


def _get_module():
    if "nc" not in _CACHE:
        _CACHE["nc"] = _build_module()
    return _CACHE["nc"]


def make_in_maps(x, w_attn, b_attn, w_proj):
    """Host-side sharding: per-core input dicts (8 cores)."""
    import ml_dtypes
    bf16 = ml_dtypes.bfloat16
    x = np.asarray(x, dtype=np.float32)
    w_attn = np.asarray(w_attn, dtype=np.float32)
    b_attn = np.asarray(b_attn, dtype=np.float32)
    w_proj = np.asarray(w_proj, dtype=np.float32)

    # xt pre-tiled to [p, c(4) k(6) t(512)]: each 512-col chunk of all six
    # 128-row k-tiles is contiguous per partition (one efficient DMA each)
    xts = []
    for b in range(B):
        xt = np.ascontiguousarray(x[b].T)            # [768, 2048]
        xt = xt.reshape(KT, 128, 4, T // 4)          # k p c t
        xt = xt.transpose(1, 2, 0, 3).reshape(128, KT * T)
        xts.append(np.ascontiguousarray(xt).astype(bf16))
    mask = np.triu(np.ones((128, 128), np.float32)).astype(bf16)

    in_maps = []
    for c in range(8):
        b = c // G
        hg = c % G
        sl = slice(CPG * hg, CPG * (hg + 1))
        wq = w_attn[0:C][sl]          # [192, 768]
        wk = w_attn[C:2 * C][sl]
        wv = w_attn[2 * C:3 * C][sl]
        bq = b_attn[0:C][sl]
        bk = b_attn[C:2 * C][sl]
        bv = b_attn[2 * C:3 * C][sl]
        # per head h: wqkt cols h*128..+128 = [wq_h (64) | wk_h (64)]
        wqkt = np.zeros((C, HPG * 128), np.float32)
        bqk = np.zeros((128, HPG), np.float32)
        for h in range(HPG):
            wqkt[:, h * 128:h * 128 + 64] = wq[h * 64:(h + 1) * 64].T
            wqkt[:, h * 128 + 64:(h + 1) * 128] = wk[h * 64:(h + 1) * 64].T
            bqk[0:64, h] = bq[h * 64:(h + 1) * 64]
            bqk[64:128, h] = bk[h * 64:(h + 1) * 64]
        wvt = np.ascontiguousarray(wv.T).astype(bf16)             # [768, 192]
        bvb = np.ascontiguousarray(
            np.broadcast_to(bv, (128, CPG))).astype(np.float32)   # [128, 192]
        wpt = np.ascontiguousarray(w_proj[:, sl].T).astype(bf16)  # [192, 768]
        in_maps.append({
            "xt": xts[b],
            "wqkt": wqkt.astype(bf16),
            "wvt": wvt,
            "bqk": bqk,
            "bv": bvb,
            "wpt": wpt,
            "mask": mask,
        })
    return in_maps


def gather(results, b_proj):
    """Sum the 4 head-group partials per batch, add bias."""
    b_proj = np.asarray(b_proj, dtype=np.float32)
    y = np.zeros((B, T, C), np.float32)
    for c in range(8):
        y[c // G] += np.asarray(results[c]["y"], dtype=np.float32)
    y += b_proj
    return y


def run(x, w_attn, b_attn, w_proj, b_proj, trace=False, **kw):
    from concourse.bass_utils import run_bass_kernel_spmd
    nc = _get_module()
    in_maps = make_in_maps(x, w_attn, b_attn, w_proj)
    res = run_bass_kernel_spmd(nc, in_maps, list(range(8)), trace=trace, **kw)
    return gather(res.results, b_proj), res


def kernel(x, w_attn, b_attn, w_proj, b_proj):
    y, _ = run(x, w_attn, b_attn, w_proj, b_proj)
    return y
